# revision 4
# baseline (speedup 1.0000x reference)
"""Trainium2 Bass kernel for nn_AugmentationLayerV2 (crop/resize + flip/rot90 +
brightness/contrast), data-parallel over batch across 8 NeuronCores.

Strategy: per image the geometric part (bilinear crop+resize, flip, rot90) is a
separable linear map  out = A^T @ X~ @ B  where A/B are tiny per-image weight
matrices built on the host from the random params.  Rotation parity flips which
factor couples to the output row index; instead of running both parity arms on
the device (one zeroed), the host absorbs the parity: for odd rotations it
sends the TRANSPOSED image X~ = X^T and swaps A/B, so the device always runs a
single branch-free arm (SPMD across cores).

All heavy tensors (image, weight matrices, output) travel as bf16 and the
matmuls run in bf16 (full-rate PE path, fp32 PSUM accumulation); the loose
2e-2 rel-err gate leaves ample margin.  Images are sent channel-planar
[u, (c v)] so every matmul weight slice is contiguous (FWL-eligible) and all
PSUM->SBUF copies write contiguous runs.

The per-channel mean (needed for contrast) is folded in as an extra column of
A/B (row-sum weights): q_c = Asum^T X~ Bsum drops out of the stage-1 result's
last column with one extra tiny matmul per row-block.  The final per-channel
affine  out = s_c * img + t_c  is fused into the PSUM->SBUF eviction, split
across the Scalar (activation) and Vector (tensor_scalar) engines to balance
their load.
"""

import sys
import numpy as np
import ml_dtypes

sys.path.insert(0, "/opt/trn_rl_repo")

BF16 = ml_dtypes.bfloat16

B, S, C = 64, 256, 5
NCORES = 8
PER = B // NCORES
GRAY = 0.2989 + 0.5870 + 0.1140
NPIX = float(S * S)
H = S // 128  # 2 row/col blocks
M = S + 1     # 257: matrix cols = S resample weights + 1 row-sum column

_CACHE = {}


# ---------------------------------------------------------------- host math
def _resample_weights(coords):
    """[S] float32 coords -> [S, S] W with out = W @ img (axis resample)."""
    i0f = np.floor(coords)
    i0 = np.clip(i0f, 0, S - 1).astype(np.int64)
    i1 = np.clip(i0f + 1.0, 0, S - 1).astype(np.int64)
    f = (coords - i0f).astype(np.float64)
    W = np.zeros((S, S), dtype=np.float64)
    np.add.at(W, (np.arange(S), i0), 1.0 - f)
    np.add.at(W, (np.arange(S), i1), f)
    return W


def _host_matrices(off_f, b_right, c_contrast, size, docrop, flp, k):
    """Build per-image A_ext [S,257], B_ext [S,257], transpose flag, and the
    affine params alpha/beta/smul [C].

    Device computes P = A^T @ X~ @ B with X~ = X (k even) or X^T (k odd),
    q_c = Asum^T X~ Bsum, out = smul*P + (alpha*q + beta)."""
    Sf = np.float32(S)
    size_f = np.float32(size) if docrop else Sf
    if docrop:
        off0 = np.float32(np.floor(np.float32(off_f[0]) * (Sf - size_f + np.float32(1.0))))
        off1 = np.float32(np.floor(np.float32(off_f[1]) * (Sf - size_f + np.float32(1.0))))
    else:
        off0 = np.float32(0.0)
        off1 = np.float32(0.0)
    scale = np.float32(size_f / Sf)
    idx = (np.arange(S, dtype=np.float32) + np.float32(0.5)) * scale - np.float32(0.5)
    Wr = _resample_weights((idx + off0).astype(np.float32))
    Wc = _resample_weights((idx + off1).astype(np.float32))

    ar = np.arange(S)
    rev = S - 1 - ar
    k = int(k)
    flp = bool(flp)
    # even k: out[i,j] = sum_{u,v} X[u,v] * Wr[pr].T[u,i] * Wc[pc].T[v,j]
    # odd  k: out[i,j] = sum_{u,v} X[u,v] * Wc[pc].T[v,i] * Wr[pr].T[u,j]
    #       = sum_{v,u} X^T[v,u] * A[v,i] * B[u,j]  with A=Wc[pc].T, B=Wr[pr].T
    if k in (0, 2):
        pr = ar if k == 0 else rev            # row perm as a function of i
        pb = ar if k == 0 else rev            # col perm as a function of j
        pc = rev[pb] if flp else pb
        A = Wr[pr].T                          # [u, i]
        Bm = Wc[pc].T                         # [v, j]
        xpose = False
    else:
        pr = ar if k == 1 else rev            # row perm as a function of j
        pb = rev if k == 1 else ar            # col perm as a function of i
        pc = rev[pb] if flp else pb
        A = Wc[pc].T                          # [v, i]
        Bm = Wr[pr].T                         # [u, j]
        xpose = True

    A_ext = np.zeros((S, M))
    A_ext[:, :S] = A
    A_ext[:, S] = A.sum(axis=1)
    B_ext = np.zeros((S, M))
    B_ext[:, :S] = Bm
    B_ext[:, S] = Bm.sum(axis=1)

    alpha = GRAY * (1.0 - c_contrast.astype(np.float64)) / NPIX   # [C]
    beta = GRAY * b_right.astype(np.float64)                      # [C]
    smul = GRAY * c_contrast.astype(np.float64)                   # [C]
    return (A_ext.astype(np.float32), B_ext.astype(np.float32), xpose,
            alpha.astype(np.float32), beta.astype(np.float32),
            smul.astype(np.float32))


# ---------------------------------------------------------------- device code
def _build_nc():
    import concourse.bacc as bacc
    import concourse.mybir as mybir
    from concourse import tile
    from contextlib import ExitStack

    f32 = mybir.dt.float32
    bf16 = mybir.dt.bfloat16
    Copy = mybir.ActivationFunctionType.Copy
    Ident = mybir.ActivationFunctionType.Identity
    MUL = mybir.AluOpType.mult
    ADD = mybir.AluOpType.add

    nc = bacc.Bacc(None, target_bir_lowering=False)
    # X: channel-planar [u, (c v)]; already host-transposed for odd rotations
    X = nc.declare_dram_parameter("X", [PER, S, S * C], bf16, isOutput=False)
    A = nc.declare_dram_parameter("A", [PER, S, M], bf16, isOutput=False)
    Bm = nc.declare_dram_parameter("B", [PER, S, M], bf16, isOutput=False)
    AB = nc.declare_dram_parameter("AB", [1, PER * 2 * C], f32, isOutput=False)
    SM = nc.declare_dram_parameter("SM", [128, PER * C], f32, isOutput=False)
    ONE = nc.declare_dram_parameter("ONE", [1, 128], f32, isOutput=False)
    # OUT: channel-planar [i, (c j)]; host de-interleaves
    OUT = nc.declare_dram_parameter("OUT", [PER, S, S * C], bf16, isOutput=True)

    with tile.TileContext(nc) as tc, ExitStack() as ctx:
        xp = ctx.enter_context(tc.tile_pool(name="xp", bufs=2 * H + 2))
        mp = ctx.enter_context(tc.tile_pool(name="mp", bufs=2 * H + 4))
        ip = ctx.enter_context(tc.tile_pool(name="ip", bufs=2 * H + 2))
        fpool = ctx.enter_context(tc.tile_pool(name="fp", bufs=2 * H))
        sp = ctx.enter_context(tc.tile_pool(name="sp", bufs=8))
        ps_i = ctx.enter_context(tc.tile_pool(name="psi", bufs=2, space="PSUM"))
        ps_p = ctx.enter_context(tc.tile_pool(name="psp", bufs=4, space="PSUM"))
        ps_s = ctx.enter_context(tc.tile_pool(name="pss", bufs=1, space="PSUM"))

        ones_t = sp.tile([1, 128], f32, tag="ones")
        nc.sync.dma_start(ones_t[:], ONE[0:1, :])
        ab_t = sp.tile([1, PER * 2 * C], f32, tag="ab")
        nc.sync.dma_start(ab_t[:], AB[0:1, :])
        sm_t = sp.tile([128, PER * C], f32, tag="sm")
        nc.sync.dma_start(sm_t[:], SM[:, :])

        for b in range(PER):
            xt = []
            for h in range(H):
                t = xp.tile([128, S * C], bf16, tag="x")
                nc.sync.dma_start(t[:], X[b, 128 * h:128 * (h + 1), :])
                xt.append(t)
            at, bt = [], []
            for h in range(H):
                t = mp.tile([128, M], bf16, tag="a")
                nc.sync.dma_start(t[:], A[b, 128 * h:128 * (h + 1), :])
                at.append(t)
                t = mp.tile([128, M], bf16, tag="b")
                nc.sync.dma_start(t[:], Bm[b, 128 * h:128 * (h + 1), :])
                bt.append(t)

            q_ps = ps_s.tile([1, C], f32, tag="q")

            # ---- stage 1: Int[v, (c m)] = sum_u X[u, (c v)] A[u, m] ----
            int_all = []
            for vb in range(H):
                it = ip.tile([128, C * M], bf16, tag="int")
                for c in range(C):
                    int_ps = ps_i.tile([128, M], f32, tag="ipsum")
                    for ub in range(H):
                        lhs = xt[ub][:, S * c + 128 * vb:S * c + 128 * (vb + 1)]
                        nc.tensor.matmul(int_ps[:], lhs, at[ub][:],
                                         start=(ub == 0), stop=(ub == H - 1))
                    dst = it[:, M * c:M * (c + 1)]
                    if c % 2 == 0:
                        nc.vector.tensor_copy(dst, int_ps[:])
                    else:
                        nc.scalar.activation(dst, int_ps[:], Copy)
                int_all.append(it)
                # mean partials: q[c] += Int[:, (c, S)]^T @ Bsum
                nc.tensor.matmul(
                    q_ps[:],
                    bt[vb][:, S:S + 1],
                    it.rearrange("p (c m) -> p c m", m=M)[:, :, S],
                    start=(vb == 0), stop=(vb == H - 1),
                    skip_group_check=True)

            # ---- per-channel bias t_c = alpha_c * q_c + beta_c, bcast ----
            trow = sp.tile([1, C], f32, tag="trow")
            nc.vector.tensor_mul(trow[:], q_ps[:],
                                 ab_t[0:1, 2 * C * b:2 * C * b + C])
            trow2 = sp.tile([1, C], f32, tag="trow2")
            nc.vector.tensor_add(trow2[:], trow[:],
                                 ab_t[0:1, 2 * C * b + C:2 * C * b + 2 * C])
            t_ps = ps_s.tile([128, C], f32, tag="tbc")
            nc.tensor.matmul(t_ps[:], ones_t[:], trow2[:], start=True, stop=True)
            tS = sp.tile([128, C], f32, tag="tS")
            nc.scalar.activation(tS[:], t_ps[:], Copy)

            # ---- stage 2: P[i, j] = sum_v Int[v, (c i)] B[v, j]; fused affine ----
            f_t = []
            for ib in range(H):
                t = fpool.tile([128, S * C], bf16, tag="f")
                f_t.append(t)
            for c in range(C):
                for ib in range(H):
                    p_ps = ps_p.tile([128, S], f32, tag="ppsum")
                    for vb in range(H):
                        nc.tensor.matmul(
                            p_ps[:],
                            int_all[vb][:, M * c + 128 * ib:M * c + 128 * (ib + 1)],
                            bt[vb][:, 0:S],
                            start=(vb == 0), stop=(vb == H - 1))
                    dst = f_t[ib][:, S * c:S * (c + 1)]
                    if c % 2 == 0:
                        nc.scalar.activation(dst, p_ps[:], Ident,
                                             bias=tS[:, c:c + 1],
                                             scale=sm_t[:, C * b + c:C * b + c + 1])
                    else:
                        nc.vector.tensor_scalar(
                            dst, p_ps[:],
                            sm_t[:, C * b + c:C * b + c + 1],
                            tS[:, c:c + 1],
                            MUL, ADD)
            for ib in range(H):
                nc.sync.dma_start(OUT[b, 128 * ib:128 * (ib + 1), :], f_t[ib][:])
    if not nc.is_finalized():
        nc.finalize()
    return nc


def _get_nc():
    if "nc" not in _CACHE:
        _CACHE["nc"] = _build_nc()
    return _CACHE["nc"]


# ---------------------------------------------------------------- entry point
def _prep_inputs(crops, off_frac, bright, contrast, crop_size, do_crop, flip, rot_k):
    """Build the 8 per-core input maps."""
    crops = np.ascontiguousarray(crops, dtype=np.float32)
    off_frac = np.asarray(off_frac)
    bright = np.asarray(bright)
    contrast = np.asarray(contrast)
    crop_size = np.asarray(crop_size)
    do_crop = np.asarray(do_crop)
    flip = np.asarray(flip)
    rot_k = np.asarray(rot_k)

    As = np.empty((B, S, M), np.float32)
    Bs = np.empty((B, S, M), np.float32)
    ABs = np.empty((B, 2 * C), np.float32)
    SMs = np.empty((B, C), np.float32)
    xposes = np.empty(B, np.bool_)
    for b in range(B):
        a, bm, xp, al, be, sm = _host_matrices(
            off_frac[b], bright[b], contrast[b], crop_size[b],
            do_crop[b], flip[b], rot_k[b])
        As[b], Bs[b], xposes[b] = a, bm, xp
        ABs[b, :C] = al
        ABs[b, C:] = be
        SMs[b] = sm

    # channel-planar bf16 images: X[b, u, (c v)] = crops~[b, u, v, c]
    Xp = np.empty((B, S, C, S), BF16)
    ev = ~xposes
    if ev.any():
        Xp[ev] = np.moveaxis(crops[ev], 3, 2)          # [n, u, c, v]
    if xposes.any():
        Xp[xposes] = crops[xposes].transpose(0, 2, 3, 1)  # [n, v, c, u]
    Xp = Xp.reshape(B, S, S * C)
    As16 = As.astype(BF16)
    Bs16 = Bs.astype(BF16)

    ones = np.ones((1, 128), np.float32)
    in_maps = []
    for core in range(NCORES):
        sl = slice(core * PER, (core + 1) * PER)
        in_maps.append({
            "X": Xp[sl], "A": As16[sl], "B": Bs16[sl],
            "AB": ABs[sl].reshape(1, PER * 2 * C),
            "SM": np.broadcast_to(SMs[sl].reshape(1, PER * C),
                                  (128, PER * C)).copy(),
            "ONE": ones,
        })
    return in_maps


def kernel(crops, off_frac, bright, contrast, crop_size, do_crop, flip, rot_k,
           _want_results=False, _trace=False):
    from concourse.bass_utils import run_bass_kernel_spmd

    nc = _get_nc()
    in_maps = _prep_inputs(crops, off_frac, bright, contrast, crop_size,
                           do_crop, flip, rot_k)
    res = run_bass_kernel_spmd(nc, in_maps, list(range(NCORES)), trace=_trace)
    out = np.empty((B, S, S, C), np.float32)
    for core in range(NCORES):
        o = res.results[core]["OUT"].reshape(PER, S, C, S)
        out[core * PER:(core + 1) * PER] = (
            o.transpose(0, 1, 3, 2).astype(np.float32))
    if _want_results:
        return out, res
    return out


# revision 17
# speedup vs baseline: 1.3681x; 1.3681x over previous
"""Trainium2 Bass kernel for nn_AugmentationLayerV2 (crop/resize + flip/rot90 +
brightness/contrast), data-parallel over batch across 8 NeuronCores.

Strategy: per image the geometric part (bilinear crop+resize, flip, rot90) is a
separable linear map  out = A^T @ X~ @ B  where A/B are tiny per-image weight
matrices built on the host from the random params.  Rotation parity flips which
factor couples to the output row index; instead of running both parity arms on
the device (one zeroed), the host absorbs the parity: for odd rotations it
sends the TRANSPOSED image X~ = X^T and swaps A/B, so the device always runs a
single branch-free arm (SPMD across cores).

The per-channel mean (needed for contrast) is a linear functional of the
inputs, q_c = Asum^T X~_c Bsum, so the host computes it exactly and folds it
into the per-channel affine  out = s_c * P + t_c; the device never touches it.
The affine is fused into the stage-2 PSUM->SBUF eviction.

All heavy tensors (image, weight matrices, output) travel as bf16 and the
matmuls run in bf16 (full-rate PE path, fp32 PSUM accumulation); the loose
2e-2 rel-err gate leaves ample margin (bf16 end-to-end measures ~6e-3).
Images are sent channel-planar [u, (c v)] so every matmul weight slice is
contiguous (FWL-eligible) and every PSUM eviction writes contiguous runs; the
image and both matrices ride in ONE input DMA per 128-row block.  Stage-1
pairs two channels per 512-wide PSUM bank and stage-2 pairs the two output
row-blocks of a channel, so evictions are few and wide, alternating between
the Scalar (activation) and Vector (tensor_scalar) engines.  Output DMAs are
dispatched from the Activation queue so input prefetch backpressure on the SP
queue cannot head-of-line block them.
"""

import sys
import numpy as np
import ml_dtypes

sys.path.insert(0, "/opt/trn_rl_repo")

BF16 = ml_dtypes.bfloat16

B, S, C = 64, 256, 5
NCORES = 8
PER = B // NCORES
GRAY = 0.2989 + 0.5870 + 0.1140
NPIX = float(S * S)
H = S // 128   # 2 row/col blocks
XW = S * C     # 1280: planar image row
TW = XW + 2 * S  # 1792: X | A | B fused input row

_CACHE = {}


# ---------------------------------------------------------------- host math
def _resample_weights(coords):
    """[S] float32 coords -> [S, S] W with out = W @ img (axis resample)."""
    i0f = np.floor(coords)
    i0 = np.clip(i0f, 0, S - 1).astype(np.int64)
    i1 = np.clip(i0f + 1.0, 0, S - 1).astype(np.int64)
    f = (coords - i0f).astype(np.float64)
    W = np.zeros((S, S), dtype=np.float64)
    np.add.at(W, (np.arange(S), i0), 1.0 - f)
    np.add.at(W, (np.arange(S), i1), f)
    return W


def _host_matrices(off_f, b_right, c_contrast, size, docrop, flp, k):
    """Build per-image A [S,S], B [S,S], transpose flag, and the affine
    params alpha/beta/smul [C].

    Device computes P = A^T @ X~ @ B with X~ = X (k even) or X^T (k odd);
    host computes q_c = Asum^T X~_c Bsum and out = smul*P + (alpha*q + beta)."""
    Sf = np.float32(S)
    size_f = np.float32(size) if docrop else Sf
    if docrop:
        off0 = np.float32(np.floor(np.float32(off_f[0]) * (Sf - size_f + np.float32(1.0))))
        off1 = np.float32(np.floor(np.float32(off_f[1]) * (Sf - size_f + np.float32(1.0))))
    else:
        off0 = np.float32(0.0)
        off1 = np.float32(0.0)
    scale = np.float32(size_f / Sf)
    idx = (np.arange(S, dtype=np.float32) + np.float32(0.5)) * scale - np.float32(0.5)
    Wr = _resample_weights((idx + off0).astype(np.float32))
    Wc = _resample_weights((idx + off1).astype(np.float32))

    ar = np.arange(S)
    rev = S - 1 - ar
    k = int(k)
    flp = bool(flp)
    # even k: out[i,j] = sum_{u,v} X[u,v] * Wr[pr].T[u,i] * Wc[pc].T[v,j]
    # odd  k: out[i,j] = sum_{u,v} X[u,v] * Wc[pc].T[v,i] * Wr[pr].T[u,j]
    #       = sum_{v,u} X^T[v,u] * A[v,i] * B[u,j]  with A=Wc[pc].T, B=Wr[pr].T
    if k in (0, 2):
        pr = ar if k == 0 else rev            # row perm as a function of i
        pb = ar if k == 0 else rev            # col perm as a function of j
        pc = rev[pb] if flp else pb
        A = Wr[pr].T                          # [u, i]
        Bm = Wc[pc].T                         # [v, j]
        xpose = False
    else:
        pr = ar if k == 1 else rev            # row perm as a function of j
        pb = rev if k == 1 else ar            # col perm as a function of i
        pc = rev[pb] if flp else pb
        A = Wc[pc].T                          # [v, i]
        Bm = Wr[pr].T                         # [u, j]
        xpose = True

    alpha = GRAY * (1.0 - c_contrast.astype(np.float64)) / NPIX   # [C]
    beta = GRAY * b_right.astype(np.float64)                      # [C]
    smul = GRAY * c_contrast.astype(np.float64)                   # [C]
    return (A.astype(np.float32), Bm.astype(np.float32), xpose,
            alpha, beta, smul.astype(np.float32))


# ---------------------------------------------------------------- device code
def _build_nc():
    import concourse.bacc as bacc
    import concourse.mybir as mybir
    from concourse import tile
    from contextlib import ExitStack

    f32 = mybir.dt.float32
    bf16 = mybir.dt.bfloat16
    Ident = mybir.ActivationFunctionType.Identity
    MUL = mybir.AluOpType.mult
    ADD = mybir.AluOpType.add

    nc = bacc.Bacc(None, target_bir_lowering=False)
    # fused input: per image, per 128-row block: [X planar (1280) | A (256) | B (256)]
    XAB = nc.declare_dram_parameter("XAB", [PER, S, TW], bf16, isOutput=False)
    # per-channel affine: [smul rows (PER*C) | t rows (PER*C)], broadcast over partitions
    STT = nc.declare_dram_parameter("STT", [128, 2 * PER * C], f32, isOutput=False)
    # OUT: channel-planar [i, (c j)]; host de-interleaves
    OUT = nc.declare_dram_parameter("OUT", [PER, S, XW], bf16, isOutput=True)

    with tile.TileContext(nc) as tc, ExitStack() as ctx:
        xp = ctx.enter_context(tc.tile_pool(name="xp", bufs=6 * H))
        ip = ctx.enter_context(tc.tile_pool(name="ip", bufs=6))
        fpool = ctx.enter_context(tc.tile_pool(name="fp", bufs=6))
        sp = ctx.enter_context(tc.tile_pool(name="sp", bufs=1))
        ps_i = ctx.enter_context(tc.tile_pool(name="psi", bufs=4, space="PSUM"))
        ps_p = ctx.enter_context(tc.tile_pool(name="psp", bufs=4, space="PSUM"))

        stt = sp.tile([128, 2 * PER * C], f32, tag="stt")

        # ---- PE warm-up: the HAM clock gate holds the PE at half rate
        # until ~3.4us of sustained activity.  Dummy matmuls during the
        # initial DMA ramp warm it so the real matmuls start at full rate.
        warm = sp.tile([128, 128], bf16, tag="warm")
        nc.vector.memset(warm[:], 0)
        warm_ps = ps_i.tile([128, 512], f32, tag="ipsum")
        for _ in range(140):
            nc.tensor.matmul(warm_ps[:, 0:8], warm[:], warm[:, 0:8],
                             start=True, stop=True, skip_group_check=True)

        def stage1(b):
            xab = []
            for h in range(H):
                t = xp.tile([128, TW], bf16, tag="x")
                nc.sync.dma_start(t[:], XAB[b, 128 * h:128 * (h + 1), :])
                xab.append(t)
            at = [t[:, XW:XW + S] for t in xab]

            # Int[v, (vb c m)] = sum_u X[u, (c v)] A[u, m]; two channels
            # share one 512-wide PSUM bank -> one wide eviction
            it = ip.tile([128, H * C * S], bf16, tag="int")
            ev = 0
            for vb in range(H):
                for c0 in range(0, C, 2):
                    nch = min(2, C - c0)
                    int_ps = ps_i.tile([128, 512], f32, tag="ipsum")
                    for c in range(c0, c0 + nch):
                        off = 256 * (c - c0)
                        for ub in range(H):
                            lhs = xab[ub][:, S * c + 128 * vb:S * c + 128 * (vb + 1)]
                            nc.tensor.matmul(int_ps[:, off:off + S], lhs, at[ub],
                                             start=(ub == 0), stop=(ub == H - 1),
                                             skip_group_check=True)
                    dst = it[:, C * S * vb + S * c0:C * S * vb + S * (c0 + nch)]
                    src = int_ps[:, 0:S * nch]
                    # stage-1 evictions live on ACT only: the DVE queue is
                    # reserved for stage-2 evictions so neither queue mixes
                    # work gated on different pipeline depths.
                    nc.scalar.activation(dst, src, Ident)
                    ev += 1
            return xab, it

        def stage2(b, xab, it, last=False):
            # P[i, j] = sum_v Int[v, (c i)] B[v, j]; fused affine.  Both
            # 128-row blocks (ib) of a channel share one 512-wide PSUM
            # bank -> single wide eviction per channel, on DVE (ACT is
            # reserved for stage-1).  In the pipeline drain (last images)
            # there is no stage-1 work left, so split across both engines.
            bt = [t[:, XW + S:XW + 2 * S] for t in xab]
            ft = fpool.tile([128, H * XW], bf16, tag="f")
            ft4 = ft.rearrange("p (h c j) -> p h c j", h=H, c=C)
            for c in range(C):
                p_ps = ps_p.tile([128, H * S], f32, tag="ppsum")
                for ib in range(H):
                    for vb in range(H):
                        nc.tensor.matmul(
                            p_ps[:, S * ib:S * (ib + 1)],
                            it[:, C * S * vb + S * c + 128 * ib:
                               C * S * vb + S * c + 128 * (ib + 1)],
                            bt[vb],
                            start=(vb == 0), stop=(vb == H - 1),
                            skip_group_check=True)
                dst = ft4[:, :, c, :]
                sm_ap = stt[:, C * b + c:C * b + c + 1]
                tt_ap = stt[:, PER * C + C * b + c:PER * C + C * b + c + 1]
                if last and c in (1, 3):
                    nc.scalar.activation(dst, p_ps[:], Ident,
                                         bias=tt_ap, scale=sm_ap)
                else:
                    nc.vector.tensor_scalar(dst, p_ps[:], sm_ap, tt_ap, MUL, ADD)
            for ib in range(H):
                nc.sync.dma_start(OUT[b, 128 * ib:128 * (ib + 1), :],
                                  ft[:, XW * ib:XW * (ib + 1)])

        # software pipeline with 2-image skew: stage-2 of image b-2 is
        # emitted after stage-1 of image b, so stage-1 evictions complete
        # well before the PE needs them as stage-2 weights.  The STT DMA
        # (only needed by the first stage-2 eviction) is issued after the
        # first image's input DMAs to keep the ramp short.
        SKEW = 2
        pending = []
        for b in range(PER):
            cur = stage1(b)
            if b == 0:
                nc.sync.dma_start(stt[:], STT[:, :])
            pending.append((b, cur))
            if len(pending) >= SKEW + 1:
                pb, pc = pending.pop(0)
                stage2(pb, *pc)
        for pb, pc in pending:
            stage2(pb, *pc, last=True)
    if not nc.is_finalized():
        nc.finalize()
    return nc


def _get_nc():
    if "nc" not in _CACHE:
        _CACHE["nc"] = _build_nc()
    return _CACHE["nc"]


# ---------------------------------------------------------------- entry point
def _prep_inputs(crops, off_frac, bright, contrast, crop_size, do_crop, flip, rot_k):
    """Build the 8 per-core input maps."""
    crops = np.ascontiguousarray(crops, dtype=np.float32)
    off_frac = np.asarray(off_frac)
    bright = np.asarray(bright)
    contrast = np.asarray(contrast)
    crop_size = np.asarray(crop_size)
    do_crop = np.asarray(do_crop)
    flip = np.asarray(flip)
    rot_k = np.asarray(rot_k)

    XABs = np.empty((B, S, TW), BF16)
    STTs = np.empty((B, 2, C), np.float32)
    xab_f = np.empty((S, TW), np.float32)
    for b in range(B):
        A, Bm, xpose, al, be, sm = _host_matrices(
            off_frac[b], bright[b], contrast[b], crop_size[b],
            do_crop[b], flip[b], rot_k[b])
        img = crops[b]
        if xpose:
            img = img.transpose(1, 0, 2)
        xpl = np.moveaxis(img, 2, 1)                      # [u, c, v] planar
        # exact per-channel mean numerator: q_c = Asum^T X~_c Bsum
        q = np.einsum("u,ucv,v->c", A.sum(axis=1, dtype=np.float64),
                      xpl.astype(np.float64), Bm.sum(axis=1, dtype=np.float64))
        t = al * q + be                                    # [C] float64
        STTs[b, 0] = sm
        STTs[b, 1] = t.astype(np.float32)
        xab_f[:, :XW] = xpl.reshape(S, XW)
        xab_f[:, XW:XW + S] = A
        xab_f[:, XW + S:] = Bm
        XABs[b] = xab_f

    in_maps = []
    for core in range(NCORES):
        sl = slice(core * PER, (core + 1) * PER)
        stt = np.concatenate([STTs[sl, 0].reshape(-1), STTs[sl, 1].reshape(-1)])
        in_maps.append({
            "XAB": XABs[sl],
            "STT": np.broadcast_to(stt, (128, 2 * PER * C)).copy(),
        })
    return in_maps


def kernel(crops, off_frac, bright, contrast, crop_size, do_crop, flip, rot_k,
           _want_results=False, _trace=False):
    from concourse.bass_utils import run_bass_kernel_spmd

    nc = _get_nc()
    in_maps = _prep_inputs(crops, off_frac, bright, contrast, crop_size,
                           do_crop, flip, rot_k)
    res = run_bass_kernel_spmd(nc, in_maps, list(range(NCORES)), trace=_trace)
    out = np.empty((B, S, S, C), np.float32)
    for core in range(NCORES):
        o = res.results[core]["OUT"].reshape(PER, S, C, S)
        out[core * PER:(core + 1) * PER] = (
            o.transpose(0, 1, 3, 2).astype(np.float32))
    if _want_results:
        return out, res
    return out
